# revision 4
# baseline (speedup 1.0000x reference)
"""Baseline TreeLSTM kernel (pre-session), reconstructed for A/B timing."""

import numpy as np

import concourse.bass as bass
import concourse.mybir as mybir
import concourse.tile as tile
from concourse import bacc
from concourse import bass_utils

S, B, D = 128, 512, 256
NCORES = 8
BS = B // NCORES          # 64 batch rows per core
NE = S * BS               # 8192 state rows per core
GD = 5 * D                # 1280 recurrent gate outputs (u,i,lf,rf,o)
XD = 4 * D                # 1024 xp outputs (cx,ix,fx,ox)
NMC = GD // 128           # 10 gate chunks
XMC = XD // 128           # 8 xp chunks
RW = 6 * 128              # state-table row: h(2x128) | c_hi(2x128) | c_lo(2x128) fp16
# psum gate chunk -> xp chunk (rf reuses the f projection)
XP_MAP10 = [0, 1, 2, 3, 4, 5, 4, 5, 6, 7]

BF16 = mybir.dt.float16
F32 = mybir.dt.float32
I16 = mybir.dt.int16
AF = mybir.ActivationFunctionType
OP = mybir.AluOpType

_CACHED = {}


def build_program():
    nc = bacc.Bacc("TRN2", target_bir_lowering=False, debug=False)

    d_xTh = nc.dram_tensor("xTh", [128, 2, NE], BF16, kind="ExternalInput").ap()
    d_xTl = nc.dram_tensor("xTl", [128, 2, NE], BF16, kind="ExternalInput").ap()
    d_w2 = nc.dram_tensor("w2", [128, 4 * NMC, 128], BF16, kind="ExternalInput").ap()
    d_wxh = nc.dram_tensor("wxh", [128, 2 * XMC, 128], BF16, kind="ExternalInput").ap()
    d_wxl = nc.dram_tensor("wxl", [128, 2 * XMC, 128], BF16, kind="ExternalInput").ap()
    d_bx = nc.dram_tensor("bx8", [128, XMC], F32, kind="ExternalInput").ap()
    d_gidx = nc.dram_tensor("gidx", [128, S, 8], I16, kind="ExternalInput").ap()
    d_ident = nc.dram_tensor("ident", [128, 128], BF16, kind="ExternalInput").ap()
    d_prevm = nc.dram_tensor("prevm", [1, S * 2 * BS], I16, kind="ExternalInput").ap()
    d_mask = nc.dram_tensor("maskv", [1, NE], BF16, kind="ExternalInput").ap()
    tbl = nc.dram_tensor("hT", [NE, RW], BF16, kind="ExternalOutput").ap()

    with tile.TileContext(nc) as tc:
        with tc.tile_pool(name="dram", bufs=1, space="DRAM") as dpool:
            xp_blocks = [dpool.tile([2 * XMC, 128, 8 * BS], BF16, name=f"xpb{i}",
                                    tag=f"xpb{i}") for i in range(16)]

            phA_cm = tc.tile_pool(name="phA", bufs=1)
            phA = phA_cm.__enter__()
            psA_cm = tc.tile_pool(name="psA", bufs=1, space="PSUM")
            psA = psA_cm.__enter__()
            stA_cm = tc.tile_pool(name="stA", bufs=4)
            stA = stA_cm.__enter__()
            stB_cm = tc.tile_pool(name="stB", bufs=2)
            stB = stB_cm.__enter__()
            xbA_cm = tc.tile_pool(name="xbA", bufs=2)
            xbA = xbA_cm.__enter__()
            s_wxh = phA.tile([128, 2 * XMC, 128], BF16)
            s_wxl = phA.tile([128, 2 * XMC, 128], BF16)
            s_bx = phA.tile([128, XMC], F32)
            nc.sync.dma_start(out=s_wxh[:], in_=d_wxh[:])
            nc.sync.dma_start(out=s_wxl[:], in_=d_wxl[:])
            nc.sync.dma_start(out=s_bx[:], in_=d_bx[:])

            NCH = 16
            CW = NE // NCH

            def emit_xp_block(nch):
                xh = xbA.tile([128, 2, CW], BF16, name=f"xh{nch}", tag="xh")
                xl = xbA.tile([128, 2, CW], BF16, name=f"xl{nch}", tag="xl")
                nc.sync.dma_start(out=xh[:], in_=d_xTh[:, :, nch * CW:(nch + 1) * CW])
                nc.sync.dma_start(out=xl[:], in_=d_xTl[:, :, nch * CW:(nch + 1) * CW])
                big = stB.tile([128, 2 * XMC, CW], BF16, name=f"big{nch}", tag="big")
                for mc in range(XMC):
                    pst = psA.tile([128, CW], F32, name=f"pstA{nch}_{mc}", tag="pstA")
                    first = True
                    for kc in range(2):
                        for wmat, xmat in ((s_wxh, xh), (s_wxl, xh), (s_wxh, xl)):
                            nc.tensor.matmul(
                                pst[:],
                                lhsT=wmat[:, mc * 2 + kc, :],
                                rhs=xmat[:, kc, :],
                                start=first,
                                stop=(kc == 1 and xmat is xl),
                            )
                            first = False
                    stg = stA.tile([128, CW], F32, name=f"stg{nch}_{mc}", tag="stg")
                    if mc % 2 == 0:
                        nc.vector.tensor_scalar_add(stg[:], pst[:], s_bx[:, mc:mc + 1])
                    else:
                        nc.scalar.activation(stg[:], pst[:], AF.Identity,
                                             bias=s_bx[:, mc:mc + 1])
                    nc.scalar.copy(big[:, mc, :], stg[:])
                    nc.vector.tensor_tensor(out=big[:, XMC + mc, :], in0=stg[:],
                                            in1=big[:, mc, :], op=OP.subtract)
                for q in range(4):
                    nc.sync.dma_start(
                        out=xp_blocks[nch][q * 4:(q + 1) * 4].rearrange(
                            "t p e -> p t e"),
                        in_=big[:, q * 4:(q + 1) * 4, :],
                    )

            emit_xp_block(0)
            emit_xp_block(1)

            import contextlib
            _pstack = contextlib.ExitStack()
            persist = _pstack.enter_context(tc.tile_pool(name="persist", bufs=1))
            s_w2 = persist.tile([128, 4 * NMC, 128], BF16)
            s_gidx = persist.tile([128, S, 8], I16)
            s_ident = persist.tile([128, 128], BF16)
            s_mask = persist.tile([128, NE], F32)
            s_prevm = persist.tile([128, S, 2 * BS], I16)

            nc.sync.dma_start(out=s_w2[:], in_=d_w2[:])
            nc.sync.dma_start(out=s_gidx[:], in_=d_gidx[:])
            nc.sync.dma_start(out=s_ident[:], in_=d_ident[:])
            mask_bcast = bass.AP(
                tensor=d_mask.tensor,
                offset=d_mask.offset,
                ap=[[0, 128]] + list(d_mask.ap[1:]),
            )
            nc.gpsimd.dma_start(out=s_mask[:], in_=mask_bcast)
            prevm_bcast = bass.AP(
                tensor=d_prevm.tensor,
                offset=d_prevm.offset,
                ap=[[0, 128]] + list(d_prevm.ap[1:]),
            )
            nc.gpsimd.dma_start(
                out=s_prevm[:].rearrange("p s j -> p (s j)"), in_=prevm_bcast)

            with (
                tc.tile_pool(name="gpool", bufs=3) as gpool,
                tc.tile_pool(name="xpool", bufs=8) as xpool,
                tc.tile_pool(name="gate", bufs=2) as gate,
                tc.tile_pool(name="psB", bufs=1, space="PSUM") as psB,
                tc.tile_pool(name="psT", bufs=1, space="PSUM") as psT,
            ):
                gbuf_nxt = None
                for s in range(S):
                    if s % 8 == 0 and s // 8 + 2 < NCH:
                        emit_xp_block(s // 8 + 2)
                    xpt = xpool.tile([128, 2, XMC, BS], BF16, tag="xpt")
                    nc.sync.dma_start(
                        out=xpt[:].rearrange("p h t b -> p (h t) b"),
                        in_=xp_blocks[s // 8][:, :, (s % 8) * BS:(s % 8 + 1) * BS]
                        .rearrange("t p b -> p t b"),
                    )

                    if s == 0:
                        gbuf = gpool.tile([128, 6, 2 * BS], BF16, tag="gbuf")
                        nc.vector.memset(gbuf[:], 0.0)
                    else:
                        gbuf = gbuf_nxt
                    if s + 1 < S:
                        gbuf_nxt = gpool.tile([128, 6, 2 * BS], BF16, tag="gbuf")
                        if s >= 1:
                            nc.gpsimd.dma_gather(
                                gbuf_nxt[:], tbl[0:s * BS, :], s_gidx[:, s + 1, :],
                                num_idxs=2 * BS, num_idxs_reg=2 * BS,
                                elem_size=RW, transpose=True,
                            )

                    mrow = s_mask[:, s * BS:(s + 1) * BS]
                    mb = bass.AP(
                        tensor=mrow.tensor,
                        offset=mrow.offset,
                        ap=[mrow.ap[0], [0, 2]] + list(mrow.ap[1:]),
                    )
                    lcrc = gate.tile([128, 2, 2 * BS], F32, tag="lcrc")
                    nc.vector.tensor_add(lcrc[:], gbuf[:, 2:4, :], gbuf[:, 4:6, :])
                    lc = lcrc[:, :, 0:BS]
                    rc = lcrc[:, :, BS:2 * BS]

                    tg = {}
                    cn = gate.tile([128, 2 * BS], F32, tag="cn")
                    t2 = gate.tile([128, 2 * BS], F32, tag="t2")
                    t3 = gate.tile([128, 2 * BS], F32, tag="t3")
                    tc_t = gate.tile([128, 2 * BS], F32, tag="tc_t")
                    cn_m = gate.tile([128, 2 * BS], F32, tag="cn_m")
                    st6 = gate.tile([128, 6, BS], BF16, tag="st6")
                    for g in range(5):
                        psg = psB.tile([128, 2 * BS], F32, name=f"psg{g}_{s}",
                                       tag=f"psg{g}")
                        for mc2 in range(2):
                            mc = g * 2 + mc2
                            for kc in range(4):
                                lr, dhi = divmod(kc, 2)
                                nc.tensor.matmul(
                                    psg[:, mc2 * BS:(mc2 + 1) * BS],
                                    lhsT=s_w2[:, mc * 4 + kc, :],
                                    rhs=gbuf[:, dhi, lr * BS:(lr + 1) * BS],
                                    start=(kc == 0),
                                    stop=False,
                                )
                            xc = XP_MAP10[mc]
                            for hl in range(2):
                                nc.tensor.matmul(
                                    psg[:, mc2 * BS:(mc2 + 1) * BS],
                                    lhsT=s_ident[:],
                                    rhs=xpt[:, hl, xc, :],
                                    start=False,
                                    stop=(hl == 1),
                                )
                        t = gate.tile([128, 2 * BS], F32, name=f"tg{g}_{s}",
                                      tag=f"tg{g}")
                        nc.scalar.activation(
                            t[:], psg[:], AF.Tanh if g == 0 else AF.Sigmoid)
                        tg[g] = t
                        if g == 1:
                            nc.vector.tensor_mul(cn[:], tg[1][:], tg[0][:])
                        elif g == 2:
                            nc.vector.tensor_mul(t2[:], tg[2][:], lc)
                            nc.vector.tensor_add(cn[:], cn[:], t2[:])
                        elif g == 3:
                            nc.vector.tensor_mul(t3[:], tg[3][:], rc)
                            nc.vector.tensor_add(cn[:], cn[:], t3[:])
                            nc.vector.tensor_tensor(
                                out=cn_m[:].rearrange("p (c b) -> p c b", c=2),
                                in0=cn[:].rearrange("p (c b) -> p c b", c=2),
                                in1=mb, op=OP.mult,
                            )
                            nc.scalar.copy(st6[:, 2:4, :], cn_m[:])
                            nc.vector.tensor_tensor(
                                out=st6[:, 4:6, :], in0=cn_m[:],
                                in1=st6[:, 2:4, :], op=OP.subtract,
                            )
                            nc.scalar.activation(tc_t[:], cn[:], AF.Tanh)
                    hn = gate.tile([128, 2 * BS], F32, tag="hn")
                    nc.vector.tensor_mul(hn[:], tg[4][:], tc_t[:])
                    nc.vector.tensor_tensor(
                        out=st6[:, 0:2, :],
                        in0=hn[:].rearrange("p (c b) -> p c b", c=2),
                        in1=mb, op=OP.mult,
                    )
                    if s + 1 < S:
                        pmrow = s_prevm[:, s + 1, :]

                        def dup_ap(c0, c1):
                            st6c = st6[:, c0:c1, :]
                            d = bass.AP(
                                tensor=st6c.tensor, offset=st6c.offset,
                                ap=[st6c.ap[0], st6c.ap[1], [0, 2], st6c.ap[2]],
                            )
                            m = bass.AP(
                                tensor=pmrow.tensor, offset=pmrow.offset,
                                ap=[pmrow.ap[0], [0, c1 - c0],
                                    [pmrow.ap[1][0] * BS, 2],
                                    [pmrow.ap[1][0], BS]],
                            )
                            return d, m

                        dh, mh = dup_ap(0, 2)
                        nc.vector.copy_predicated(
                            out=gbuf_nxt[:, 0:2, :].rearrange(
                                "p c (l b) -> p c l b", l=2),
                            mask=mh, data=dh,
                        )
                        dc, mc_ = dup_ap(2, 6)
                        nc.vector.copy_predicated(
                            out=gbuf_nxt[:, 2:6, :].rearrange(
                                "p c (l b) -> p c l b", l=2),
                            mask=mc_, data=dc,
                        )
                    pstA = psT.tile([BS, 4, 128], F32, tag="pstA")
                    pstB = psT.tile([BS, 2, 128], F32, tag="pstB")
                    for ch in (2, 3, 4, 5, 0, 1):
                        dst = pstA[:, ch - 2, :] if ch >= 2 else pstB[:, ch, :]
                        nc.tensor.matmul(
                            dst,
                            lhsT=st6[:, ch, :],
                            rhs=s_ident[:],
                            start=True, stop=True,
                        )
                    stage = gate.tile([BS, 6, 128], BF16, tag="stage")
                    nc.scalar.copy(stage[:, 2:6, :], pstA[:])
                    nc.scalar.copy(stage[:, 0:2, :], pstB[:])
                    nc.sync.dma_start(
                        out=tbl[s * BS:(s + 1) * BS, :],
                        in_=stage[:].rearrange("b c p -> b (c p)"),
                    )
            _pstack.close()
            xbA_cm.__exit__(None, None, None)
            stB_cm.__exit__(None, None, None)
            stA_cm.__exit__(None, None, None)
            psA_cm.__exit__(None, None, None)
            phA_cm.__exit__(None, None, None)
    nc.compile()
    return nc


def _prep_core_inputs(x, x_mask, li, ri, Wx, bx, Wlh, Wrh, core):
    b0 = core * BS

    xr = x[:, b0:b0 + BS, :]
    xT = np.ascontiguousarray(xr.transpose(2, 0, 1))
    xT = xT.reshape(2, 128, NE).transpose(1, 0, 2)
    xT = np.ascontiguousarray(xT).astype(np.float32)
    xTh = xT.astype(np.float16)
    xTl = (xT - xTh.astype(np.float32)).astype(np.float16)

    lidx = li[:, b0:b0 + BS]
    ridx = ri[:, b0:b0 + BS]
    steps = np.arange(S)[:, None]
    is_prev = np.concatenate([lidx == steps - 1, ridx == steps - 1], axis=1)
    prevm = np.zeros((1, S * 2 * BS), np.int16)
    prevm[0] = is_prev.astype(np.int16).reshape(-1)

    lif = lidx * BS + np.arange(BS)[None, :]
    rif = ridx * BS + np.arange(BS)[None, :]
    flat = np.concatenate([lif, rif], axis=1)
    flat = np.where(is_prev, np.tile(np.arange(BS), 2)[None, :], flat)
    gidx = np.zeros((128, S, 8), np.int16)
    for j in range(2 * BS):
        gidx[np.arange(128) % 16 == (j % 16), :, j // 16] = flat[:, j][None, :]

    maskv = np.ascontiguousarray(
        x_mask[:, b0:b0 + BS].reshape(1, NE)).astype(np.float16)

    return {"xTh": xTh, "xTl": xTl, "gidx": gidx, "maskv": maskv,
            "prevm": prevm}


def _prep_shared_inputs(Wx, bx, Wlh, Wrh):
    bf16 = np.float16
    W2 = np.zeros((2 * D, GD), np.float32)
    for g in range(5):
        W2[:D, g * D:(g + 1) * D] = Wlh[g].T
        W2[D:, g * D:(g + 1) * D] = Wrh[g].T
    w2 = np.zeros((128, 4 * NMC, 128), np.float32)
    for mc in range(NMC):
        for kc in range(4):
            w2[:, mc * 4 + kc, :] = W2[kc * 128:(kc + 1) * 128,
                                       mc * 128:(mc + 1) * 128]
    WxM = np.zeros((D, XD), np.float32)
    for g in range(4):
        WxM[:, g * D:(g + 1) * D] = Wx[g].T
    wx = np.zeros((128, 2 * XMC, 128), np.float32)
    for mc in range(XMC):
        for kc in range(2):
            wx[:, mc * 2 + kc, :] = WxM[kc * 128:(kc + 1) * 128,
                                        mc * 128:(mc + 1) * 128]
    bxf = bx.reshape(XD)
    bx8 = np.zeros((128, XMC), np.float32)
    for mc in range(XMC):
        bx8[:, mc] = bxf[mc * 128:(mc + 1) * 128]
    wxh = wx.astype(np.float16)
    wxl = (wx - wxh.astype(np.float32)).astype(np.float16)
    return {"w2": w2.astype(bf16), "wxh": wxh, "wxl": wxl, "bx8": bx8,
            "ident": np.eye(128, dtype=np.float16)}


def kernel(x, x_mask, x_left_mask, x_right_mask, Wx, bx, Wlh, Wrh):
    x = np.asarray(x, np.float32)
    x_mask = np.asarray(x_mask, np.float32)
    li = np.argmax(np.asarray(x_left_mask), axis=-1).astype(np.int64)
    ri = np.argmax(np.asarray(x_right_mask), axis=-1).astype(np.int64)
    Wx = np.asarray(Wx, np.float32)
    bx = np.asarray(bx, np.float32)
    Wlh = np.asarray(Wlh, np.float32)
    Wrh = np.asarray(Wrh, np.float32)

    if "nc" not in _CACHED:
        _CACHED["nc"] = build_program()
    nc = _CACHED["nc"]

    shared = _prep_shared_inputs(Wx, bx, Wlh, Wrh)
    in_maps = []
    for core in range(NCORES):
        m = _prep_core_inputs(x, x_mask, li, ri, Wx, bx, Wlh, Wrh, core)
        m.update(shared)
        in_maps.append(m)

    res = bass_utils.run_bass_kernel_spmd(nc, in_maps, core_ids=list(range(NCORES)))
    _CACHED["last_results"] = res

    out = np.empty((B, S, D), np.float32)
    for core in range(NCORES):
        hT = np.asarray(res.results[core]["hT"])
        h = hT[:, :D].astype(np.float32).reshape(S, BS, D)
        out[core * BS:(core + 1) * BS] = h.transpose(1, 0, 2)
    return out
